# revision 3
# baseline (speedup 1.0000x reference)
"""Ragged-segment attention for Trainium2 (8 NeuronCores, SPMD), bin-dense fp16.

Per-segment masking/softmax structure is folded into a host-built low-rank
additive mask applied with ONE matmul per bin:
    mask[q,k] = (kb[k] + NEG) * 1  +  sum_s (-NEG) * 1_s[q] 1_s[k]
so scores/softmax/exp-transpose/out are all dense [128 x 128] bin ops and
segments pack at arbitrary offsets (first-fit decreasing, ~97% dense bins).

DMAs are batched per 4-bin group (context, masks, outputs) because each DMA
instruction costs ~625ns of serialized HWDGE descriptor-generation time.
"""
import numpy as np

import concourse.bacc as bacc
import concourse.mybir as mybir
import concourse.tile as tile
from concourse.bass_utils import run_bass_kernel_spmd

F32 = mybir.dt.float32
F32R = mybir.dt.float32r
FP16 = mybir.dt.float16

N_CORES = 8
D = 512
BIN = 128
GROUP = 4

LAST_RESULTS = {}


def _plan(lengths, mode):
    S = len(lengths)
    n_slots = S // N_CORES
    order = np.argsort(-lengths, kind="stable")
    seg_ids = [[int(order[N_CORES * j + c]) for j in range(n_slots)]
               for c in range(N_CORES)]
    if mode == "f32r":
        slot_len = [min(128, -(-int(lengths[order[N_CORES * j]]) // 2) * 2)
                    for j in range(n_slots)]
    else:
        slot_len = [int(lengths[order[N_CORES * j]]) for j in range(n_slots)]

    bins = []   # (used-token count, n_segs) per bin
    slots = []  # (bin, off, L)
    for j, L in enumerate(slot_len):
        bi = next((i for i, (used, ns) in enumerate(bins)
                   if used + L <= BIN and ns < 31), None)
        if bi is None:
            bins.append((0, 0))
            bi = len(bins) - 1
        used, ns = bins[bi]
        slots.append((bi, used, L))
        bins[bi] = (used + L, ns + 1)
    n_bins = ((len(bins) + GROUP - 1) // GROUP) * GROUP
    return slots, n_bins, seg_ids


def _mask_layout(slots, n_bins):
    by_bin = [[] for _ in range(n_bins)]
    for bi, off, L in slots:
        by_bin[bi].append((off, L))
    kmask = [len(by_bin[b]) + 1 for b in range(n_bins)]
    assert max(kmask) <= 32
    return by_bin, kmask


def _build(slots, n_bins, mode, repeat=1, out_fp16=None):
    DT = F32R if mode == "f32r" else FP16
    if out_fp16 is None:
        out_fp16 = (mode == "fp16")
    ODT = FP16 if out_fp16 else F32
    NPDT = np.float32 if mode == "f32r" else np.float16
    nc = bacc.Bacc("TRN2", target_bir_lowering=False)
    T = n_bins * BIN
    n_groups = n_bins // GROUP

    by_bin, kmask = _mask_layout(slots, n_bins)

    cpk = nc.dram_tensor("cpk", [T, D], DT, kind="ExternalInput")
    wt = nc.dram_tensor("wt", [128, 4 * D], DT, kind="ExternalInput")
    bvec = nc.dram_tensor("bvec", [128, 4], F32, kind="ExternalInput")
    # per-group mask rows: bin i of a group at partitions [32i, 32i+km)
    msk = nc.dram_tensor("msk", [n_groups * 128, 2 * 128], DT,
                         kind="ExternalInput")
    opk = nc.dram_tensor("opk", [T, D], ODT, kind="ExternalOutput")

    ident = nc.inline_tensor(np.eye(128, dtype=NPDT), name="ident")

    with tile.TileContext(nc) as tc:
        with (
            tc.tile_pool(name="const", bufs=1) as cpool,
            tc.tile_pool(name="cb", bufs=3) as cbp,
            tc.tile_pool(name="grp", bufs=3) as grp,
            tc.tile_pool(name="seg", bufs=4) as segp,
            tc.tile_pool(name="stat", bufs=6) as statp,
            tc.tile_pool(name="outp", bufs=2) as outp,
            tc.tile_pool(name="mk", bufs=3) as mkp,
            tc.tile_pool(name="ups", bufs=2, space="PSUM") as ups,
            tc.tile_pool(name="scps", bufs=2, space="PSUM") as scps,
            tc.tile_pool(name="trps", bufs=2, space="PSUM") as trps,
            tc.tile_pool(name="teps", bufs=1, space="PSUM") as teps,
            tc.tile_pool(name="ops", bufs=1, space="PSUM") as opsp,
        ):
            wt_sb = cpool.tile([128, 4, D], DT, tag="wt")
            b_sb = cpool.tile([128, 4], F32, tag="b")
            id_t = cpool.tile([128, 128], DT, tag="id")
            nc.sync.dma_start(wt_sb[:], wt.ap().rearrange("p (c e) -> p c e", c=4))
            nc.sync.dma_start(b_sb[:], bvec[:])
            nc.sync.dma_start(id_t[:], ident[:] if mode != "f32r"
                              else ident.ap().bitcast(F32R))

            cpk_v = cpk.ap().rearrange("(b p) d -> p b d", p=BIN)
            opk_v = opk.ap().rearrange("(b p) d -> p b d", p=BIN)
            msk_v = msk.ap().rearrange("(g r) (t p) -> g r t p", t=2, g=n_groups)

            def load_group(g):
                """DMA in context+masks for group g."""
                cg = cbp.tile([128, GROUP, D], DT, tag="cg")
                nc.sync.dma_start(
                    cg[:], cpk_v[:, g * GROUP:(g + 1) * GROUP, :])
                mg = mkp.tile([128, 2, 128], DT, tag="mg")
                nc.sync.dma_start(mg[:], msk_v[g])
                return cg, mg

            def transpose_bin(st, i):
                cg, ct = st["cg"], st["ct"]
                for k in range(4):
                    pt = trps.tile([128, 128], DT, tag="tr")
                    nc.tensor.transpose(
                        pt[:], cg[:, i, k * 128:(k + 1) * 128], id_t[:])
                    nc.vector.tensor_copy(ct[:, k, i, :], pt[:])

            def transpose_group_dma(st):
                # fp16 only: xbar DMA-transpose straight from DRAM
                g, ct = st["g"], st["ct"]
                for k in range(4):
                    nc.sync.dma_start_transpose(
                        ct[:, k, :, :],
                        cpk[g * GROUP * BIN:(g + 1) * GROUP * BIN,
                            k * 128:(k + 1) * 128])

            def u_chunk(st, c):
                ct, ut = st["ct"], st["ut"]
                ups_t = ups.tile([128, GROUP * 128], F32, tag="ups")
                for k in range(4):
                    nc.tensor.matmul(
                        ups_t[:], wt_sb[:, k, c * 128:(c + 1) * 128],
                        ct[:, k, :, :], start=(k == 0), stop=(k == 3))
                nc.scalar.activation(
                    ut[:, c, :, :], ups_t[:],
                    mybir.ActivationFunctionType.Tanh, bias=b_sb[:, c:c + 1])

            def bin_scores(st, i):
                g = st["g"]
                b = g * GROUP + i
                if not by_bin[b]:
                    return
                ct, ut, mg = st["ct"], st["ut"], st["mg"]
                km = kmask[b]
                sc = scps.tile([128, 128], F32, tag="sc")
                for k in range(4):
                    nc.tensor.matmul(
                        sc[:], ct[:, k, i, :], ut[:, k, i, :],
                        start=(k == 0), stop=False)
                nc.tensor.matmul(sc[:], mg[32 * i:32 * i + km, 0, :],
                                 mg[32 * i:32 * i + km, 1, :],
                                 start=False, stop=True,
                                 tile_position=(32 * i, 0))

                nmax = statp.tile([128, 1], F32, tag="nmax")
                sums = statp.tile([128, 1], F32, tag="sums")
                recip = statp.tile([128, 1], F32, tag="recip")
                expt = segp.tile([128, 128], DT, tag="expt")
                nc.vector.tensor_reduce(
                    nmax[:], sc[:], axis=mybir.AxisListType.X,
                    op=mybir.AluOpType.max, negate=True)
                nc.scalar.activation(
                    expt[:], sc[:], mybir.ActivationFunctionType.Exp,
                    bias=nmax[:], accum_out=sums[:])
                nc.vector.reciprocal(recip[:], sums[:])
                st[("bin", i)] = (expt, recip)

            def bin_out(st, i, use_act_copy):
                if ("bin", i) not in st:
                    return
                expt, recip = st.pop(("bin", i))
                cg, og = st["cg"], st["og"]
                tp = teps.tile([128, 128], DT, tag="te")
                nc.tensor.transpose(tp[:], expt[:], id_t[:])
                attn = segp.tile([128, 128], DT, tag="attn")
                nc.vector.tensor_copy(attn[:], tp[:])

                ops_t = opsp.tile([128, D], F32, tag="ops")
                nc.tensor.matmul(ops_t[:], attn[:], cg[:, i, :],
                                 start=True, stop=True)
                # normalize rows by 1/sum during the psum->sbuf copy
                if use_act_copy:
                    nc.scalar.activation(og[:, i, :], ops_t[:],
                                         mybir.ActivationFunctionType.Copy,
                                         scale=recip[:])
                else:
                    nc.vector.tensor_scalar_mul(og[:, i, :], ops_t[:], recip[:])

            def store_group(st):
                g = st["g"]
                # POOL/SWDGE queue: keeps the blocking store off the SP
                # load queue (SP DMAs issue in order)
                nc.gpsimd.dma_start(
                    opk_v[:, g * GROUP:(g + 1) * GROUP, :], st["og"])

            # software pipeline over groups: while group g's bins run their
            # softmax chains, interleave group g+1's transposes and u-matmuls
            # into the PE stream so the (in-order) PE never idles.
            niter = repeat * n_groups
            states = {}
            for it in range(niter + 1):
                if it < niter:
                    g = it % n_groups
                    cg, mg = load_group(g)
                    ct_t = grp.tile([128, 4, GROUP, 128], DT, tag="ct")
                    ut_t = grp.tile([128, 4, GROUP, 128], DT, tag="ut")
                    og_t = outp.tile([128, GROUP, D], ODT, tag="og")
                    st_new = {"g": g, "cg": cg, "mg": mg,
                              "ct": ct_t, "ut": ut_t, "og": og_t}
                else:
                    st_new = None
                st_old = states.pop(it - 1, None)

                prev = None
                for i in range(GROUP):
                    if st_new is not None:
                        transpose_bin(st_new, i)
                    if st_old is not None:
                        bin_scores(st_old, i)
                        if prev is not None:
                            bin_out(st_old, prev, use_act_copy=(prev % 2 == 0))
                        prev = i
                for c in range(4):
                    if st_new is not None:
                        u_chunk(st_new, c)
                if st_old is not None:
                    if prev is not None:
                        bin_out(st_old, prev, use_act_copy=(prev % 2 == 0))
                    store_group(st_old)
                if st_new is not None:
                    states[it] = st_new

    nc.compile()
    return nc


def _host_arrays(slots, n_bins, seg_ids, lengths, context, W, b, mode,
                 out_fp16=None):
    DT = np.float32 if mode == "f32r" else np.float16
    NEG = -1.0e30 if mode == "f32r" else -30000.0
    T = n_bins * BIN
    by_bin2 = [[] for _ in range(n_bins)]
    for j, (bi, off, L) in enumerate(slots):
        by_bin2[bi].append((j, off, L))
    n_groups = n_bins // GROUP

    wt = np.ascontiguousarray(
        W.T.reshape(4, 128, D).transpose(1, 0, 2).reshape(128, 4 * D)).astype(DT)
    bvec = np.ascontiguousarray(b.reshape(4, 128).T).astype(np.float32)

    in_maps = []
    for c in range(N_CORES):
        cpk = np.zeros((T, D), DT)
        kb = np.full(T, NEG, np.float32)
        for j, (bi, off, _L) in enumerate(slots):
            s = seg_ids[c][j]
            n = int(lengths[s])
            r0 = bi * BIN + off
            cpk[r0:r0 + n] = context[s, :n].astype(DT)
            kb[r0:r0 + n] = 0.0
        msk = np.zeros((n_groups * 128, 2, 128), np.float32)
        for bb in range(n_bins):
            r0 = (bb // GROUP) * 128 + 32 * (bb % GROUP)
            msk[r0, 0] = 1.0
            msk[r0, 1] = kb[bb * BIN:(bb + 1) * BIN] + NEG
            for r, (_j, off, L) in enumerate(by_bin2[bb]):
                msk[r0 + 1 + r, 0, off:off + L] = 1.0
                msk[r0 + 1 + r, 1, off:off + L] = -NEG
        in_maps.append({"cpk": cpk, "wt": wt, "bvec": bvec,
                        "msk": msk.reshape(n_groups * 128, 256).astype(DT)})
    return in_maps


_CACHE = {}


def kernel(context, lengths, W, b, mode="fp16"):
    context = np.asarray(context, dtype=np.float32)
    lengths = np.asarray(lengths, dtype=np.int32)
    W = np.asarray(W, dtype=np.float32)
    b = np.asarray(b, dtype=np.float32)
    S, Lmax, Din = context.shape

    slots, n_bins, seg_ids = _plan(lengths, mode)
    key = (tuple(slots), n_bins, mode)
    if key in _CACHE:
        nc = _CACHE[key]
    else:
        nc = _build(slots, n_bins, mode)
        _CACHE[key] = nc

    in_maps = _host_arrays(slots, n_bins, seg_ids, lengths, context, W, b, mode)
    res = run_bass_kernel_spmd(nc, in_maps, list(range(N_CORES)))
    LAST_RESULTS["exec_time_ns"] = res.exec_time_ns

    out = np.zeros((S, Lmax, D), np.float32)
    for c in range(N_CORES):
        opk = res.results[c]["opk"].astype(np.float32)
        for j, (bi, off, _L) in enumerate(slots):
            s = seg_ids[c][j]
            n = int(lengths[s])
            r0 = bi * BIN + off
            out[s, :n] = opk[r0:r0 + n]
    return out


# revision 4
# speedup vs baseline: 1.4354x; 1.4354x over previous
"""Ragged-segment attention for Trainium2 (8 NeuronCores, SPMD), bin-dense fp16.

Per-segment masking/softmax structure is folded into a host-built low-rank
additive mask applied with ONE matmul per bin:
    mask[q,k] = (kb[k] + NEG) * 1  +  sum_s (-NEG) * 1_s[q] 1_s[k]
so scores/softmax/exp-transpose/out are all dense [128 x 128] bin ops and
segments pack at arbitrary offsets (first-fit decreasing, ~97% dense bins).

DMAs are batched per 4-bin group (context, masks, outputs) because each DMA
instruction costs ~625ns of serialized HWDGE descriptor-generation time.
"""
import numpy as np

import concourse.bacc as bacc
import concourse.mybir as mybir
import concourse.tile as tile
from concourse.bass_utils import run_bass_kernel_spmd

F32 = mybir.dt.float32
F32R = mybir.dt.float32r
FP16 = mybir.dt.float16

N_CORES = 8
D = 512
BIN = 128
GROUP = 4

LAST_RESULTS = {}


def _plan(lengths, mode):
    S = len(lengths)
    n_slots = S // N_CORES
    order = np.argsort(-lengths, kind="stable")
    seg_ids = [[int(order[N_CORES * j + c]) for j in range(n_slots)]
               for c in range(N_CORES)]
    if mode == "f32r":
        slot_len = [min(128, -(-int(lengths[order[N_CORES * j]]) // 2) * 2)
                    for j in range(n_slots)]
    else:
        slot_len = [int(lengths[order[N_CORES * j]]) for j in range(n_slots)]

    bins = []   # (used-token count, n_segs) per bin
    slots = []  # (bin, off, L)
    for j, L in enumerate(slot_len):
        bi = next((i for i, (used, ns) in enumerate(bins)
                   if used + L <= BIN and ns < 31), None)
        if bi is None:
            bins.append((0, 0))
            bi = len(bins) - 1
        used, ns = bins[bi]
        slots.append((bi, used, L))
        bins[bi] = (used + L, ns + 1)
    n_bins = ((len(bins) + GROUP - 1) // GROUP) * GROUP
    return slots, n_bins, seg_ids


def _mask_layout(slots, n_bins):
    by_bin = [[] for _ in range(n_bins)]
    for bi, off, L in slots:
        by_bin[bi].append((off, L))
    kmask = [len(by_bin[b]) + 1 for b in range(n_bins)]
    assert max(kmask) <= 32
    return by_bin, kmask


def _build(slots, n_bins, mode, repeat=1, out_fp16=None):
    DT = F32R if mode == "f32r" else FP16
    if out_fp16 is None:
        out_fp16 = (mode == "fp16")
    ODT = FP16 if out_fp16 else F32
    NPDT = np.float32 if mode == "f32r" else np.float16
    nc = bacc.Bacc("TRN2", target_bir_lowering=False)
    T = n_bins * BIN
    n_groups = n_bins // GROUP

    by_bin, kmask = _mask_layout(slots, n_bins)

    cpk = nc.dram_tensor("cpk", [T, D], DT, kind="ExternalInput")
    wt = nc.dram_tensor("wt", [128, 4 * D], DT, kind="ExternalInput")
    bvec = nc.dram_tensor("bvec", [128, 4], F32, kind="ExternalInput")
    # per-group mask rows: bin i of a group at partitions [32i, 32i+km)
    msk = nc.dram_tensor("msk", [n_groups * 128, 2 * 128], DT,
                         kind="ExternalInput")
    opk = nc.dram_tensor("opk", [T, D], ODT, kind="ExternalOutput")

    ident = nc.inline_tensor(np.eye(128, dtype=NPDT), name="ident")

    with tile.TileContext(nc) as tc:
        with (
            tc.tile_pool(name="const", bufs=1) as cpool,
            tc.tile_pool(name="cb", bufs=3) as cbp,
            tc.tile_pool(name="grp", bufs=3) as grp,
            tc.tile_pool(name="seg", bufs=4) as segp,
            tc.tile_pool(name="stat", bufs=6) as statp,
            tc.tile_pool(name="outp", bufs=2) as outp,
            tc.tile_pool(name="mk", bufs=3) as mkp,
            tc.tile_pool(name="ups", bufs=2, space="PSUM") as ups,
            tc.tile_pool(name="scps", bufs=2, space="PSUM") as scps,
            tc.tile_pool(name="trps", bufs=2, space="PSUM") as trps,
            tc.tile_pool(name="teps", bufs=1, space="PSUM") as teps,
            tc.tile_pool(name="ops", bufs=1, space="PSUM") as opsp,
        ):
            wt_sb = cpool.tile([128, 4, D], DT, tag="wt")
            b_sb = cpool.tile([128, 4], F32, tag="b")
            id_t = cpool.tile([128, 128], DT, tag="id")
            nc.sync.dma_start(wt_sb[:], wt.ap().rearrange("p (c e) -> p c e", c=4))
            nc.sync.dma_start(b_sb[:], bvec[:])
            nc.sync.dma_start(id_t[:], ident[:] if mode != "f32r"
                              else ident.ap().bitcast(F32R))

            cpk_v = cpk.ap().rearrange("(b p) d -> p b d", p=BIN)
            opk_v = opk.ap().rearrange("(b p) d -> p b d", p=BIN)
            msk_v = msk.ap().rearrange("(g r) (t p) -> g r t p", t=2, g=n_groups)

            def load_group(g):
                """DMA in context+masks for group g."""
                cg = cbp.tile([128, GROUP, D], DT, tag="cg")
                nc.sync.dma_start(
                    cg[:], cpk_v[:, g * GROUP:(g + 1) * GROUP, :])
                mg = mkp.tile([128, 2, 128], DT, tag="mg")
                nc.sync.dma_start(mg[:], msk_v[g])
                return cg, mg

            def transpose_bin(st, i):
                cg, ct = st["cg"], st["ct"]
                for k in range(4):
                    pt = trps.tile([128, 128], DT, tag="tr")
                    nc.tensor.transpose(
                        pt[:], cg[:, i, k * 128:(k + 1) * 128], id_t[:])
                    nc.vector.tensor_copy(ct[:, k, i, :], pt[:])

            def transpose_group_dma(st):
                # fp16 only: xbar DMA-transpose straight from DRAM
                g, ct = st["g"], st["ct"]
                for k in range(4):
                    nc.sync.dma_start_transpose(
                        ct[:, k, :, :],
                        cpk[g * GROUP * BIN:(g + 1) * GROUP * BIN,
                            k * 128:(k + 1) * 128])

            def u_chunk(st, c):
                ct, ut = st["ct"], st["ut"]
                ups_t = ups.tile([128, GROUP * 128], F32, tag="ups")
                for k in range(4):
                    nc.tensor.matmul(
                        ups_t[:], wt_sb[:, k, c * 128:(c + 1) * 128],
                        ct[:, k, :, :], start=(k == 0), stop=(k == 3))
                nc.scalar.activation(
                    ut[:, c, :, :], ups_t[:],
                    mybir.ActivationFunctionType.Tanh, bias=b_sb[:, c:c + 1])

            def bin_scores(st, i):
                g = st["g"]
                b = g * GROUP + i
                if not by_bin[b]:
                    return
                ct, ut, mg = st["ct"], st["ut"], st["mg"]
                km = kmask[b]
                sc = scps.tile([128, 128], F32, tag="sc")
                for k in range(4):
                    nc.tensor.matmul(
                        sc[:], ct[:, k, i, :], ut[:, k, i, :],
                        start=(k == 0), stop=False)
                nc.tensor.matmul(sc[:], mg[32 * i:32 * i + km, 0, :],
                                 mg[32 * i:32 * i + km, 1, :],
                                 start=False, stop=True,
                                 tile_position=(32 * i, 0))

                nmax = statp.tile([128, 1], F32, tag="nmax")
                sums = statp.tile([128, 1], F32, tag="sums")
                recip = statp.tile([128, 1], F32, tag="recip")
                expt = segp.tile([128, 128], DT, tag="expt")
                nc.vector.tensor_reduce(
                    nmax[:], sc[:], axis=mybir.AxisListType.X,
                    op=mybir.AluOpType.max, negate=True)
                nc.scalar.activation(
                    expt[:], sc[:], mybir.ActivationFunctionType.Exp,
                    bias=nmax[:], accum_out=sums[:])
                nc.vector.reciprocal(recip[:], sums[:])
                st[("bin", i)] = (expt, recip)

            def bin_out(st, i, use_act_copy):
                if ("bin", i) not in st:
                    return
                expt, recip = st.pop(("bin", i))
                cg, og = st["cg"], st["og"]
                tp = teps.tile([128, 128], DT, tag="te")
                nc.tensor.transpose(tp[:], expt[:], id_t[:])
                attn = segp.tile([128, 128], DT, tag="attn")
                nc.vector.tensor_copy(attn[:], tp[:])

                ops_t = opsp.tile([128, D], F32, tag="ops")
                nc.tensor.matmul(ops_t[:], attn[:], cg[:, i, :],
                                 start=True, stop=True)
                # normalize rows by 1/sum during the psum->sbuf copy
                if use_act_copy:
                    nc.scalar.activation(og[:, i, :], ops_t[:],
                                         mybir.ActivationFunctionType.Copy,
                                         scale=recip[:])
                else:
                    nc.vector.tensor_scalar_mul(og[:, i, :], ops_t[:], recip[:])

            def store_group(st):
                g = st["g"]
                # ACT HWDGE queue: keeps the blocking store off the SP
                # load queue (HWDGE DMAs issue in order per engine queue)
                nc.scalar.dma_start(
                    opk_v[:, g * GROUP:(g + 1) * GROUP, :], st["og"])

            # software pipeline over groups: while group g's bins run their
            # softmax chains, interleave group g+1's transposes and u-matmuls
            # into the PE stream so the (in-order) PE never idles.
            niter = repeat * n_groups
            states = {}
            for it in range(niter + 1):
                if it < niter:
                    g = it % n_groups
                    cg, mg = load_group(g)
                    ct_t = grp.tile([128, 4, GROUP, 128], DT, tag="ct")
                    ut_t = grp.tile([128, 4, GROUP, 128], DT, tag="ut")
                    og_t = outp.tile([128, GROUP, D], ODT, tag="og")
                    st_new = {"g": g, "cg": cg, "mg": mg,
                              "ct": ct_t, "ut": ut_t, "og": og_t}
                else:
                    st_new = None
                st_old = states.pop(it - 1, None)

                prev = None
                for i in range(GROUP):
                    if st_new is not None:
                        transpose_bin(st_new, i)
                    if st_old is not None:
                        bin_scores(st_old, i)
                        if prev is not None:
                            bin_out(st_old, prev, use_act_copy=(prev % 2 == 0))
                        prev = i
                for c in range(4):
                    if st_new is not None:
                        u_chunk(st_new, c)
                if st_old is not None:
                    if prev is not None:
                        bin_out(st_old, prev, use_act_copy=(prev % 2 == 0))
                    store_group(st_old)
                if st_new is not None:
                    states[it] = st_new

    nc.compile()
    return nc


def _host_arrays(slots, n_bins, seg_ids, lengths, context, W, b, mode,
                 out_fp16=None):
    DT = np.float32 if mode == "f32r" else np.float16
    NEG = -1.0e30 if mode == "f32r" else -30000.0
    T = n_bins * BIN
    by_bin2 = [[] for _ in range(n_bins)]
    for j, (bi, off, L) in enumerate(slots):
        by_bin2[bi].append((j, off, L))
    n_groups = n_bins // GROUP

    wt = np.ascontiguousarray(
        W.T.reshape(4, 128, D).transpose(1, 0, 2).reshape(128, 4 * D)).astype(DT)
    bvec = np.ascontiguousarray(b.reshape(4, 128).T).astype(np.float32)

    in_maps = []
    for c in range(N_CORES):
        cpk = np.zeros((T, D), DT)
        kb = np.full(T, NEG, np.float32)
        for j, (bi, off, _L) in enumerate(slots):
            s = seg_ids[c][j]
            n = int(lengths[s])
            r0 = bi * BIN + off
            cpk[r0:r0 + n] = context[s, :n].astype(DT)
            kb[r0:r0 + n] = 0.0
        msk = np.zeros((n_groups * 128, 2, 128), np.float32)
        for bb in range(n_bins):
            r0 = (bb // GROUP) * 128 + 32 * (bb % GROUP)
            msk[r0, 0] = 1.0
            msk[r0, 1] = kb[bb * BIN:(bb + 1) * BIN] + NEG
            for r, (_j, off, L) in enumerate(by_bin2[bb]):
                msk[r0 + 1 + r, 0, off:off + L] = 1.0
                msk[r0 + 1 + r, 1, off:off + L] = -NEG
        in_maps.append({"cpk": cpk, "wt": wt, "bvec": bvec,
                        "msk": msk.reshape(n_groups * 128, 256).astype(DT)})
    return in_maps


_CACHE = {}


def kernel(context, lengths, W, b, mode="fp16"):
    context = np.asarray(context, dtype=np.float32)
    lengths = np.asarray(lengths, dtype=np.int32)
    W = np.asarray(W, dtype=np.float32)
    b = np.asarray(b, dtype=np.float32)
    S, Lmax, Din = context.shape

    slots, n_bins, seg_ids = _plan(lengths, mode)
    key = (tuple(slots), n_bins, mode)
    if key in _CACHE:
        nc = _CACHE[key]
    else:
        nc = _build(slots, n_bins, mode)
        _CACHE[key] = nc

    in_maps = _host_arrays(slots, n_bins, seg_ids, lengths, context, W, b, mode)
    res = run_bass_kernel_spmd(nc, in_maps, list(range(N_CORES)))
    LAST_RESULTS["exec_time_ns"] = res.exec_time_ns

    out = np.zeros((S, Lmax, D), np.float32)
    for c in range(N_CORES):
        opk = res.results[c]["opk"].astype(np.float32)
        for j, (bi, off, _L) in enumerate(slots):
            s = seg_ids[c][j]
            n = int(lengths[s])
            r0 = bi * BIN + off
            out[s, :n] = opk[r0:r0 + n]
    return out


# revision 5
# speedup vs baseline: 1.6248x; 1.1320x over previous
"""Ragged-segment attention for Trainium2 (8 NeuronCores, SPMD), bin-dense fp16.

Per-segment masking/softmax structure is folded into a host-built low-rank
additive mask applied with ONE matmul per bin:
    mask[q,k] = (kb[k] + NEG) * 1  +  sum_s (-NEG) * 1_s[q] 1_s[k]
so scores/softmax/exp-transpose/out are all dense [128 x 128] bin ops and
segments pack at arbitrary offsets (first-fit decreasing, ~97% dense bins).

DMAs are batched per 4-bin group (context, masks, outputs) because each DMA
instruction costs ~625ns of serialized HWDGE descriptor-generation time.
"""
import numpy as np

import concourse.bacc as bacc
import concourse.mybir as mybir
import concourse.tile as tile
from concourse.bass_utils import run_bass_kernel_spmd

F32 = mybir.dt.float32
F32R = mybir.dt.float32r
FP16 = mybir.dt.float16

N_CORES = 8
D = 512
BIN = 128
GROUP = 4

LAST_RESULTS = {}


def _plan(lengths, mode):
    S = len(lengths)
    n_slots = S // N_CORES
    order = np.argsort(-lengths, kind="stable")
    seg_ids = [[int(order[N_CORES * j + c]) for j in range(n_slots)]
               for c in range(N_CORES)]
    if mode == "f32r":
        slot_len = [min(128, -(-int(lengths[order[N_CORES * j]]) // 2) * 2)
                    for j in range(n_slots)]
    else:
        slot_len = [int(lengths[order[N_CORES * j]]) for j in range(n_slots)]

    bins = []   # (used-token count, n_segs) per bin
    slots = []  # (bin, off, L)
    for j, L in enumerate(slot_len):
        bi = next((i for i, (used, ns) in enumerate(bins)
                   if used + L <= BIN and ns < 31), None)
        if bi is None:
            bins.append((0, 0))
            bi = len(bins) - 1
        used, ns = bins[bi]
        slots.append((bi, used, L))
        bins[bi] = (used + L, ns + 1)
    n_bins = ((len(bins) + GROUP - 1) // GROUP) * GROUP
    return slots, n_bins, seg_ids


def _mask_layout(slots, n_bins):
    by_bin = [[] for _ in range(n_bins)]
    for bi, off, L in slots:
        by_bin[bi].append((off, L))
    kmask = [len(by_bin[b]) + 1 for b in range(n_bins)]
    assert max(kmask) <= 32
    return by_bin, kmask


def _build(slots, n_bins, mode, repeat=1, out_fp16=None):
    DT = F32R if mode == "f32r" else FP16
    if out_fp16 is None:
        out_fp16 = (mode == "fp16")
    ODT = FP16 if out_fp16 else F32
    NPDT = np.float32 if mode == "f32r" else np.float16
    nc = bacc.Bacc("TRN2", target_bir_lowering=False)
    T = n_bins * BIN
    n_groups = n_bins // GROUP

    by_bin, kmask = _mask_layout(slots, n_bins)

    cpk = nc.dram_tensor("cpk", [T, D], DT, kind="ExternalInput")
    wt = nc.dram_tensor("wt", [128, 4 * D], DT, kind="ExternalInput")
    bvec = nc.dram_tensor("bvec", [128, 4], F32, kind="ExternalInput")
    # per-group mask rows: bin i of a group at partitions [32i, 32i+km)
    msk = nc.dram_tensor("msk", [n_groups * 128, 2 * 128], DT,
                         kind="ExternalInput")
    opk = nc.dram_tensor("opk", [T, D], ODT, kind="ExternalOutput")

    ident = nc.inline_tensor(np.eye(128, dtype=NPDT), name="ident")

    with tile.TileContext(nc) as tc:
        with (
            tc.tile_pool(name="const", bufs=1) as cpool,
            tc.tile_pool(name="cb", bufs=3) as cbp,
            tc.tile_pool(name="grp", bufs=3) as grp,
            tc.tile_pool(name="seg", bufs=4) as segp,
            tc.tile_pool(name="stat", bufs=6) as statp,
            tc.tile_pool(name="outp", bufs=2) as outp,
            tc.tile_pool(name="mk", bufs=3) as mkp,
            tc.tile_pool(name="ups", bufs=2, space="PSUM") as ups,
            tc.tile_pool(name="scps", bufs=2, space="PSUM") as scps,
            tc.tile_pool(name="trps", bufs=2, space="PSUM") as trps,
            tc.tile_pool(name="teps", bufs=1, space="PSUM") as teps,
            tc.tile_pool(name="ops", bufs=1, space="PSUM") as opsp,
        ):
            wt_sb = cpool.tile([128, 4, D], DT, tag="wt")
            b_sb = cpool.tile([128, 4], F32, tag="b")
            id_t = cpool.tile([128, 128], DT, tag="id")
            nc.sync.dma_start(wt_sb[:], wt.ap().rearrange("p (c e) -> p c e", c=4))
            nc.sync.dma_start(b_sb[:], bvec[:])
            nc.sync.dma_start(id_t[:], ident[:] if mode != "f32r"
                              else ident.ap().bitcast(F32R))

            cpk_v = cpk.ap().rearrange("(b p) d -> p b d", p=BIN)
            opk_v = opk.ap().rearrange("(b p) d -> p b d", p=BIN)
            msk_v = msk.ap().rearrange("(g r) (t p) -> g r t p", t=2, g=n_groups)

            def load_group(g):
                """DMA in context+masks for group g."""
                cg = cbp.tile([128, GROUP, D], DT, tag="cg")
                nc.sync.dma_start(
                    cg[:], cpk_v[:, g * GROUP:(g + 1) * GROUP, :])
                mg = mkp.tile([128, 2, 128], DT, tag="mg")
                nc.sync.dma_start(mg[:], msk_v[g])
                return cg, mg

            def transpose_bin(st, i):
                cg, ct = st["cg"], st["ct"]
                for k in range(4):
                    pt = trps.tile([128, 128], DT, tag="tr")
                    nc.tensor.transpose(
                        pt[:], cg[:, i, k * 128:(k + 1) * 128], id_t[:])
                    nc.vector.tensor_copy(ct[:, k, i, :], pt[:])

            def transpose_group_dma(st):
                # fp16 only: xbar DMA-transpose straight from DRAM
                g, ct = st["g"], st["ct"]
                for k in range(4):
                    nc.sync.dma_start_transpose(
                        ct[:, k, :, :],
                        cpk[g * GROUP * BIN:(g + 1) * GROUP * BIN,
                            k * 128:(k + 1) * 128])

            def u_chunk(st, c):
                ct, ut = st["ct"], st["ut"]
                ups_t = ups.tile([128, GROUP * 128], F32, tag="ups")
                for k in range(4):
                    nc.tensor.matmul(
                        ups_t[:], wt_sb[:, k, c * 128:(c + 1) * 128],
                        ct[:, k, :, :], start=(k == 0), stop=(k == 3))
                nc.scalar.activation(
                    ut[:, c, :, :], ups_t[:],
                    mybir.ActivationFunctionType.Tanh, bias=b_sb[:, c:c + 1])

            def bin_scores(st, i):
                g = st["g"]
                b = g * GROUP + i
                if not by_bin[b]:
                    return
                ct, ut, mg = st["ct"], st["ut"], st["mg"]
                km = kmask[b]
                sc = scps.tile([128, 128], F32, tag="sc")
                for k in range(4):
                    nc.tensor.matmul(
                        sc[:], ct[:, k, i, :], ut[:, k, i, :],
                        start=(k == 0), stop=False)
                nc.tensor.matmul(sc[:], mg[32 * i:32 * i + km, 0, :],
                                 mg[32 * i:32 * i + km, 1, :],
                                 start=False, stop=True,
                                 tile_position=(32 * i, 0))

                nmax = statp.tile([128, 1], F32, tag="nmax")
                sums = statp.tile([128, 1], F32, tag="sums")
                recip = statp.tile([128, 1], F32, tag="recip")
                expt = segp.tile([128, 128], DT, tag="expt")
                nc.vector.tensor_reduce(
                    nmax[:], sc[:], axis=mybir.AxisListType.X,
                    op=mybir.AluOpType.max, negate=True)
                nc.scalar.activation(
                    expt[:], sc[:], mybir.ActivationFunctionType.Exp,
                    bias=nmax[:], accum_out=sums[:])
                nc.vector.reciprocal(recip[:], sums[:])
                st[("bin", i)] = (expt, recip)

            def bin_out(st, i, use_act_copy):
                if ("bin", i) not in st:
                    return
                expt, recip = st.pop(("bin", i))
                cg, og = st["cg"], st["og"]
                tp = teps.tile([128, 128], DT, tag="te")
                nc.tensor.transpose(tp[:], expt[:], id_t[:])
                attn = segp.tile([128, 128], DT, tag="attn")
                nc.vector.tensor_copy(attn[:], tp[:])

                ops_t = opsp.tile([128, D], F32, tag="ops")
                nc.tensor.matmul(ops_t[:], attn[:], cg[:, i, :],
                                 start=True, stop=True)
                # normalize rows by 1/sum during the psum->sbuf copy
                if use_act_copy:
                    nc.scalar.activation(og[:, i, :], ops_t[:],
                                         mybir.ActivationFunctionType.Copy,
                                         scale=recip[:])
                else:
                    nc.vector.tensor_scalar_mul(og[:, i, :], ops_t[:], recip[:])

            def store_group(st):
                g = st["g"]
                # ACT HWDGE queue: keeps the blocking store off the SP
                # load queue (HWDGE DMAs issue in order per engine queue)
                nc.scalar.dma_start(
                    opk_v[:, g * GROUP:(g + 1) * GROUP, :], st["og"])

            # software pipeline over groups: while group g's bins run their
            # softmax chains, interleave group g+1's transposes and u-matmuls
            # into the PE stream so the (in-order) PE never idles.
            niter = repeat * n_groups
            states = {}
            for it in range(niter + 1):
                if it < niter:
                    g = it % n_groups
                    cg, mg = load_group(g)
                    ct_t = grp.tile([128, 4, GROUP, 128], DT, tag="ct")
                    ut_t = grp.tile([128, 4, GROUP, 128], DT, tag="ut")
                    og_t = outp.tile([128, GROUP, D], ODT, tag="og")
                    st_new = {"g": g, "cg": cg, "mg": mg,
                              "ct": ct_t, "ut": ut_t, "og": og_t}
                else:
                    st_new = None
                st_old = states.pop(it - 1, None)

                pend = []
                for i in range(GROUP):
                    if st_new is not None:
                        transpose_bin(st_new, i)
                    if st_old is not None:
                        bin_scores(st_old, i)
                        pend.append(i)
                        if len(pend) > 2:
                            j = pend.pop(0)
                            bin_out(st_old, j, use_act_copy=(j % 2 == 0))
                for c in range(4):
                    if st_new is not None:
                        u_chunk(st_new, c)
                if st_old is not None:
                    for j in pend:
                        bin_out(st_old, j, use_act_copy=(j % 2 == 0))
                    store_group(st_old)
                if st_new is not None:
                    states[it] = st_new

    nc.compile()
    return nc


def _host_arrays(slots, n_bins, seg_ids, lengths, context, W, b, mode,
                 out_fp16=None):
    DT = np.float32 if mode == "f32r" else np.float16
    NEG = -1.0e30 if mode == "f32r" else -30000.0
    T = n_bins * BIN
    by_bin2 = [[] for _ in range(n_bins)]
    for j, (bi, off, L) in enumerate(slots):
        by_bin2[bi].append((j, off, L))
    n_groups = n_bins // GROUP

    wt = np.ascontiguousarray(
        W.T.reshape(4, 128, D).transpose(1, 0, 2).reshape(128, 4 * D)).astype(DT)
    bvec = np.ascontiguousarray(b.reshape(4, 128).T).astype(np.float32)

    in_maps = []
    for c in range(N_CORES):
        cpk = np.zeros((T, D), DT)
        kb = np.full(T, NEG, np.float32)
        for j, (bi, off, _L) in enumerate(slots):
            s = seg_ids[c][j]
            n = int(lengths[s])
            r0 = bi * BIN + off
            cpk[r0:r0 + n] = context[s, :n].astype(DT)
            kb[r0:r0 + n] = 0.0
        msk = np.zeros((n_groups * 128, 2, 128), np.float32)
        for bb in range(n_bins):
            r0 = (bb // GROUP) * 128 + 32 * (bb % GROUP)
            msk[r0, 0] = 1.0
            msk[r0, 1] = kb[bb * BIN:(bb + 1) * BIN] + NEG
            for r, (_j, off, L) in enumerate(by_bin2[bb]):
                msk[r0 + 1 + r, 0, off:off + L] = 1.0
                msk[r0 + 1 + r, 1, off:off + L] = -NEG
        in_maps.append({"cpk": cpk, "wt": wt, "bvec": bvec,
                        "msk": msk.reshape(n_groups * 128, 256).astype(DT)})
    return in_maps


_CACHE = {}


def kernel(context, lengths, W, b, mode="fp16"):
    context = np.asarray(context, dtype=np.float32)
    lengths = np.asarray(lengths, dtype=np.int32)
    W = np.asarray(W, dtype=np.float32)
    b = np.asarray(b, dtype=np.float32)
    S, Lmax, Din = context.shape

    slots, n_bins, seg_ids = _plan(lengths, mode)
    key = (tuple(slots), n_bins, mode)
    if key in _CACHE:
        nc = _CACHE[key]
    else:
        nc = _build(slots, n_bins, mode)
        _CACHE[key] = nc

    in_maps = _host_arrays(slots, n_bins, seg_ids, lengths, context, W, b, mode)
    res = run_bass_kernel_spmd(nc, in_maps, list(range(N_CORES)))
    LAST_RESULTS["exec_time_ns"] = res.exec_time_ns

    out = np.zeros((S, Lmax, D), np.float32)
    for c in range(N_CORES):
        opk = res.results[c]["opk"].astype(np.float32)
        for j, (bi, off, _L) in enumerate(slots):
            s = seg_ids[c][j]
            n = int(lengths[s])
            r0 = bi * BIN + off
            out[s, :n] = opk[r0:r0 + n]
    return out
